# revision 4
# baseline (speedup 1.0000x reference)
"""AttnBlock (B=2, C=512, H=W=64) on 8 TRN2 NeuronCores — fp8 DoubleRow.

Sharding: core c handles batch b=c//4 and query-quarter q=c%4 (1024 of 4096
query positions). Keys/values are computed redundantly per core from the
full batch image; the key axis is host-permuted per core so the core's
query quarter occupies columns 0:1024 (softmax is permutation-invariant
over keys).

All matmuls run in fp8e4 (TRN FP8_EXP4, max 240) with DoubleRow perf mode:
each instruction contracts K=256 (two 128-row subtiles interleaved in the
free dim) in the same 512 cycles a bf16/f32r K=128 matmul takes — 2x.
Channel dim C=512 is stored pair-split as [128 p, 2 g, 2 t] with
c = p + 128*t + 256*g; a DR matmul contracts (p, t) for fixed g, and the
two g-groups accumulate in PSUM.

GroupNorm: bn_stats on the fp8 x (24 slices on DVE, 8 slices via ACT
Identity/Square accum_out), group reduce via one-hot matmuls; scale/shift
fold into the q/k/v weights on device (wq' = wq*diag(s), bias' = wq@t + b
via tiny DR matvecs), so no separate normalize pass exists. Softmax runs
unshifted with exp(s*C^-.5 - 2) to keep fp8 in range; Z comes from a
ones-stationary DR matmul accumulated alongside PV; O is normalized by
1/Z (broadcast via a K=1 outer-product matmul + full-width reciprocal)
during PSUM evacuation. The residual path stays exact fp32.
"""

import numpy as np
import ml_dtypes

import concourse.bass as bass
import concourse.tile as tile
from concourse import bacc, mybir
from concourse.bass_utils import run_bass_kernel_spmd

F32 = mybir.dt.float32
F32R = mybir.dt.float32r
F8 = mybir.dt.float8e4
E4 = ml_dtypes.float8_e4m3
DR = mybir.MatmulPerfMode.DoubleRow
AF = mybir.ActivationFunctionType

P = 128
C = 512
N = 4096          # H*W keys
NQ = 1024         # query columns per core
NS = 8            # 512-wide column slices of N
SPL = 6           # slices per (g,t) whose stats run on DVE (rest on ACT)
NJP = 16          # 256-wide key pair-tiles
B = 2
HW = 64
NGROUPS = 32
GSIZE = C // NGROUPS
EPS = 1e-5
SCL = float(C) ** -0.5
EBIAS = -2.0      # exp(s*SCL - 2): max logit ~5.5 -> exp(3.5)=33 << 240
TS = 64.0         # shift vector pre-scale for fp8 matvec
NCORES = 8

_cached = {}


def _build_program():
    nc = bacc.Bacc("TRN2", target_bir_lowering=False, debug=False)

    X8_d = nc.declare_dram_parameter("xin8", [P, 2, 2, N], F8, isOutput=False)
    W3_d = nc.declare_dram_parameter("w38", [P, 2, 2, 3, C], F8, isOutput=False)
    WP_d = nc.declare_dram_parameter("wp8", [P, 2, 2, C], F8, isOutput=False)
    # packed per-channel f32 consts: bq, bk, bv, bp, gamma, beta
    CP_d = nc.declare_dram_parameter("cpack", [P, 2, 2, 6], F32, isOutput=False)
    G_d = nc.declare_dram_parameter("gmat", [P, 2, 2, NGROUPS], F32, isOutput=False)
    E_d = nc.declare_dram_parameter("emat", [NGROUPS, 2, 2, P], F32, isOutput=False)
    ON8_d = nc.declare_dram_parameter("ones8", [P, 2, P], F8, isOutput=False)
    ONQ_d = nc.declare_dram_parameter("onesq", [1, P], F32R, isOutput=False)
    XQ_d = nc.declare_dram_parameter("xq", [P, 2, 2, NQ], F32, isOutput=False)
    OUT_d = nc.declare_dram_parameter("out", [P, 2, 2, NQ], F32, isOutput=True)

    with tile.TileContext(nc) as tc:
        with (
            tc.tile_pool(name="big", bufs=1) as big,
            tc.tile_pool(name="consts", bufs=1) as consts,
            tc.tile_pool(name="stat", bufs=1) as stat,
            tc.tile_pool(name="work", bufs=1) as work,
        ):
            X8 = big.tile([P, 2, 2, N], F8)
            K8 = big.tile([P, 2, 2, N], F8)
            VT8 = big.tile([P, NJP, 2, C], F8)
            QO8 = big.tile([P, 2, 2, NQ], F8)
            xq_sb = big.tile([P, 2, 2, NQ], F32)
            ost_sb = big.tile([P, 2, 2, NQ], F32)

            w38 = consts.tile([P, 2, 2, 3, C], F8)
            w3f = consts.tile([P, 2, 2, 3, C], F8)
            wp8 = consts.tile([P, 2, 2, C], F8)
            cpk = consts.tile([P, 2, 2, 6], F32)
            gmat = consts.tile([P, 2, 2, NGROUPS], F32)
            emat = consts.tile([NGROUPS, 2, 2, P], F32)
            on8 = consts.tile([P, 2, P], F8)
            onq = consts.tile([1, P], F32R)

            # preload ACT tables (Identity/Square/Sqrt/Exp) while DMA runs
            dummy = stat.tile([1, 2], F32)
            nc.vector.memset(dummy, 1.0)
            dscr = stat.tile([1, 2], F32)
            for fn in (AF.Identity, AF.Square, AF.Sqrt, AF.Exp):
                nc.scalar.activation(out=dscr, in_=dummy, func=fn)

            # x first: groupnorm stats are the serial head of the kernel
            for g in range(2):
                for t2 in range(2):
                    nc.sync.dma_start(out=X8[:, g, t2, :], in_=X8_d[:, g, t2, :])
            for t_ in (
                (w38, W3_d), (cpk, CP_d),
                (gmat, G_d), (emat, E_d), (on8, ON8_d), (onq, ONQ_d),
                (wp8, WP_d),
            ):
                nc.gpsimd.dma_start(out=t_[0], in_=t_[1][:])
            nc.gpsimd.dma_start(out=xq_sb, in_=XQ_d[:])

            bqh = cpk[:, :, :, 0]
            bkh = cpk[:, :, :, 1]
            bvh = cpk[:, :, :, 2]
            bph = cpk[:, :, :, 3]
            gam_sb = cpk[:, :, :, 4]
            bet_sb = cpk[:, :, :, 5]

            # ---------------- Phase 1: group-norm statistics ----------------
            # 24 slices via DVE bn_stats, 8 slices via ACT accum (sum, sumsq)
            bnst = stat.tile([P, 2, 2, SPL, 6], F32)
            asum = stat.tile([P, 2, 2, 2, 2], F32)
            ascr = stat.tile([P, 2, 512], F8)
            mex = stat.tile([P, 2, 2, 2], F32)
            for g in range(2):
                for t2 in range(2):
                    for s in range(SPL):
                        nc.vector.bn_stats(
                            out=bnst[:, g, t2, s, :],
                            in_=X8[:, g, t2, s * 512 : (s + 1) * 512],
                        )
                    nc.vector.bn_aggr(
                        out=mex[:, g, t2, :], in_=bnst[:, g, t2, :, :]
                    )
                    for si in range(2):
                        sl = slice((SPL + si) * 512, (SPL + si + 1) * 512)
                        nc.scalar.activation(
                            out=ascr[:, 0, :], in_=X8[:, g, t2, sl],
                            func=AF.Identity,
                            accum_out=asum[:, g, t2, si, 0:1],
                        )
                        nc.scalar.activation(
                            out=ascr[:, 1, :], in_=X8[:, g, t2, sl],
                            func=AF.Square,
                            accum_out=asum[:, g, t2, si, 1:2],
                        )
            # mexp[...,0] = mean over 4096, mexp[...,1] = E[x^2] over 4096
            W_DVE = SPL / float(NS)
            astot = stat.tile([P, 2, 2, 2], F32)
            nc.vector.tensor_add(
                out=astot, in0=asum[:, :, :, 0, :], in1=asum[:, :, :, 1, :]
            )
            mexp = stat.tile([P, 2, 2, 2], F32)
            t1s = stat.tile([P, 2, 2], F32)
            nc.vector.tensor_scalar(
                out=t1s, in0=mex[:, :, :, 0], scalar1=W_DVE, scalar2=None,
                op0=mybir.AluOpType.mult,
            )
            nc.vector.scalar_tensor_tensor(
                out=mexp[:, :, :, 0], in0=astot[:, :, :, 0],
                scalar=1.0 / float(N), in1=t1s,
                op0=mybir.AluOpType.mult, op1=mybir.AluOpType.add,
            )
            nc.vector.tensor_tensor(
                out=t1s, in0=mex[:, :, :, 0], in1=mex[:, :, :, 0],
                op=mybir.AluOpType.mult,
            )
            nc.vector.tensor_add(out=t1s, in0=t1s, in1=mex[:, :, :, 1])
            nc.vector.tensor_scalar(
                out=t1s, in0=t1s, scalar1=W_DVE, scalar2=None,
                op0=mybir.AluOpType.mult,
            )
            nc.vector.scalar_tensor_tensor(
                out=mexp[:, :, :, 1], in0=astot[:, :, :, 1],
                scalar=1.0 / float(N), in1=t1s,
                op0=mybir.AluOpType.mult, op1=mybir.AluOpType.add,
            )

            scale_c = stat.tile([P, 2, 2], F32)
            shift_c = stat.tile([P, 2, 2], F32)
            tv8 = stat.tile([P, 2, 2, 16], F8)
            bv8 = stat.tile([P, 2, 2, 16], F8)
            bqe = stat.tile([P, 2, 2], F32)
            bke = stat.tile([P, 2, 2], F32)
            bve = stat.tile([P, 2, 2], F32)
            bpe = stat.tile([P, 2, 2], F32)
            neg2 = stat.tile([P, 1], F32)
            nc.vector.memset(neg2, EBIAS)

            with tc.tile_pool(name="psum_p1", bufs=1, space="PSUM") as p1:
                gs_ps = p1.tile([NGROUPS, 2], F32, tag="gs")
                kk = 0
                for g in range(2):
                    for t2 in range(2):
                        nc.tensor.matmul(
                            gs_ps, gmat[:, g, t2, :], mexp[:, g, t2, :],
                            start=(kk == 0), stop=(kk == 3),
                        )
                        kk += 1
                gsb = stat.tile([NGROUPS, 2], F32)
                nc.vector.tensor_copy(out=gsb, in_=gs_ps)
                gmr = stat.tile([NGROUPS, 2], F32)
                gtmp = stat.tile([NGROUPS, 2], F32)
                nc.vector.tensor_scalar(
                    out=gmr[:, 0:1], in0=gsb[:, 0:1], scalar1=1.0 / GSIZE,
                    scalar2=None, op0=mybir.AluOpType.mult,
                )
                nc.vector.tensor_scalar(
                    out=gtmp[:, 0:1], in0=gsb[:, 1:2], scalar1=1.0 / GSIZE,
                    scalar2=None, op0=mybir.AluOpType.mult,
                )
                nc.vector.tensor_tensor(
                    out=gtmp[:, 1:2], in0=gmr[:, 0:1], in1=gmr[:, 0:1],
                    op=mybir.AluOpType.mult,
                )
                nc.vector.tensor_sub(
                    out=gtmp[:, 0:1], in0=gtmp[:, 0:1], in1=gtmp[:, 1:2]
                )
                eps_sb = stat.tile([NGROUPS, 1], F32)
                nc.vector.memset(eps_sb, EPS)
                nc.scalar.activation(
                    out=gtmp[:, 0:1], in_=gtmp[:, 0:1],
                    func=AF.Sqrt, bias=eps_sb,
                )
                nc.vector.reciprocal(out=gmr[:, 1:2], in_=gtmp[:, 0:1])
                mc = stat.tile([P, 2, 2, 2], F32)
                ms_list = []
                for g in range(2):
                    for t2 in range(2):
                        ms_ps = p1.tile(
                            [P, 2], F32, tag="ms", bufs=4, name=f"ms{g}{t2}"
                        )
                        nc.tensor.matmul(
                            ms_ps, emat[:, g, t2, :], gmr, start=True, stop=True
                        )
                        ms_list.append((g, t2, ms_ps))
                for g, t2, ms_ps in ms_list:
                    nc.vector.tensor_copy(out=mc[:, g, t2, :], in_=ms_ps)
                nc.vector.tensor_tensor(
                    out=scale_c, in0=mc[:, :, :, 1], in1=gam_sb,
                    op=mybir.AluOpType.mult,
                )
                nc.vector.tensor_tensor(
                    out=shift_c, in0=mc[:, :, :, 0], in1=scale_c,
                    op=mybir.AluOpType.mult,
                )
                nc.vector.tensor_sub(out=shift_c, in0=bet_sb, in1=shift_c)

                # fold norm scale into q/k/v weights (wq on DVE first so the
                # q-projection can start; wk/wv on ACT in parallel)
                nc.vector.tensor_scalar(
                    out=tv8[:, :, :, 0], in0=shift_c, scalar1=TS, scalar2=None,
                    op0=mybir.AluOpType.mult,
                )
                for g in range(2):
                    for t2 in range(2):
                        sc1 = scale_c[:, g, t2 : t2 + 1]
                        if t2 == 0:
                            nc.vector.tensor_scalar(
                                out=w3f[:, g, t2, :, :], in0=w38[:, g, t2, :, :],
                                scalar1=sc1, scalar2=None,
                                op0=mybir.AluOpType.mult,
                            )
                        else:
                            nc.scalar.activation(
                                out=w3f[:, g, t2, :, :], in_=w38[:, g, t2, :, :],
                                func=AF.Identity, scale=sc1,
                            )

                # effective biases: bX_eff = wX @ shift + bX (tiny DR matvecs)
                def matvec(wi, rhs8, bh, bdst):
                    for ct in range(4):
                        g2, tt = ct // 2, ct % 2
                        be_ps = p1.tile([P, 1], F32, tag="bias", bufs=3)
                        for g in range(2):
                            lhs = (
                                wp8[:, g, :, ct * P : (ct + 1) * P]
                                if wi == 3
                                else w38[:, g, :, wi, ct * P : (ct + 1) * P]
                            )
                            nc.tensor.matmul(
                                be_ps, lhs, rhs8[:, g, :, 0:1],
                                start=(g == 0), stop=(g == 1),
                                perf_mode=DR,
                            )
                        nc.vector.tensor_scalar(
                            out=bdst[:, g2, tt : tt + 1], in0=be_ps,
                            scalar1=1.0 / TS, scalar2=bh[:, g2, tt : tt + 1],
                            op0=mybir.AluOpType.mult, op1=mybir.AluOpType.add,
                        )

                matvec(0, tv8, bqh, bqe)
                matvec(1, tv8, bkh, bke)


            # ---------------- Phase 2: q/k/v projections --------------------
            ev = {"n": 0}

            def evac(dst, src_ps, bias_ap=None):
                use_act = ev["n"] % 2 == 0
                ev["n"] += 1
                if use_act:
                    nc.scalar.activation(
                        out=dst, in_=src_ps, func=AF.Identity,
                        bias=bias_ap if bias_ap is not None else 0.0,
                    )
                elif bias_ap is None:
                    nc.vector.tensor_copy(out=dst, in_=src_ps)
                else:
                    nc.vector.tensor_scalar(
                        out=dst, in0=src_ps, scalar1=bias_ap, scalar2=None,
                        op0=mybir.AluOpType.add,
                    )

            with tc.tile_pool(name="psum2", bufs=1, space="PSUM") as p2:

                def late_biases():
                    matvec2(2, tv8, bvh, bve)
                    nc.vector.tensor_scalar(
                        out=bv8[:, :, :, 0], in0=bve, scalar1=TS, scalar2=None,
                        op0=mybir.AluOpType.mult,
                    )
                    matvec2(3, bv8, bph, bpe)

                def matvec2(wi, rhs8, bh, bdst):
                    for ct in range(4):
                        g2, tt = ct // 2, ct % 2
                        be_ps = p2.tile([P, 1], F32, tag="bias", bufs=2)
                        for g in range(2):
                            lhs = (
                                wp8[:, g, :, ct * P : (ct + 1) * P]
                                if wi == 3
                                else w38[:, g, :, wi, ct * P : (ct + 1) * P]
                            )
                            nc.tensor.matmul(
                                be_ps, lhs, rhs8[:, g, :, 0:1],
                                start=(g == 0), stop=(g == 1),
                                perf_mode=DR,
                            )
                        nc.vector.tensor_scalar(
                            out=bdst[:, g2, tt : tt + 1], in0=be_ps,
                            scalar1=1.0 / TS, scalar2=bh[:, g2, tt : tt + 1],
                            op0=mybir.AluOpType.mult, op1=mybir.AluOpType.add,
                        )

                for s in range(NS):
                    sl = slice(s * 512, (s + 1) * 512)
                    if s < 2:
                        for ct in range(4):
                            g2, tt = ct // 2, ct % 2
                            qp = p2.tile([P, 512], F32, tag="acc", bufs=3)
                            for g in range(2):
                                nc.tensor.matmul(
                                    qp,
                                    w3f[:, g, :, 0, ct * P : (ct + 1) * P],
                                    X8[:, g, :, sl],
                                    start=(g == 0), stop=(g == 1),
                                    perf_mode=DR,
                                )
                            evac(QO8[:, g2, tt, sl], qp, bqe[:, g2, tt : tt + 1])
                    for jt4 in range(4):
                        jt = s * 4 + jt4
                        jb = slice(jt * P, (jt + 1) * P)
                        vp = p2.tile([P, 512], F32, tag="acc", bufs=3)
                        for g in range(2):
                            nc.tensor.matmul(
                                vp, X8[:, g, :, jb], w3f[:, g, :, 2, :],
                                start=(g == 0), stop=(g == 1),
                                perf_mode=DR,
                            )
                        evac(VT8[:, jt // 2, jt % 2, :], vp)
                    for ct in range(4):
                        g2, tt = ct // 2, ct % 2
                        kp = p2.tile([P, 512], F32, tag="acc", bufs=3)
                        for g in range(2):
                            nc.tensor.matmul(
                                kp,
                                w3f[:, g, :, 1, ct * P : (ct + 1) * P],
                                X8[:, g, :, sl],
                                start=(g == 0), stop=(g == 1),
                                perf_mode=DR,
                            )
                        evac(K8[:, g2, tt, sl], kp, bke[:, g2, tt : tt + 1])
                    if s == 0:
                        late_biases()
                        nc.scalar.activation(out=dscr, in_=dummy, func=AF.Exp)

            # ---------------- Phase 3: attention -----------------------------
            # isl 0's output projection + epilogue are interleaved into
            # isl 1's jp loop (PSUM tag "zb" hosts zbc then the pr tiles).
            deferred = []

            def pop_deferred():
                if deferred:
                    deferred.pop(0)()

            with tc.tile_pool(name="psum3", bufs=1, space="PSUM") as p3:

                def proj_epilogue(isl, ct, O8, zbcS, p3=p3):
                    g2, tt = ct // 2, ct % 2
                    isl_sl = slice(isl * 512, (isl + 1) * 512)
                    tag = "zb" if isl == 0 else f"o{ct}"
                    pr = p3.tile([P, 512], F32, tag=tag, bufs=1, name=f"pr{isl}{ct}")
                    for g in range(2):
                        nc.tensor.matmul(
                            pr,
                            wp8[:, g, :, ct * P : (ct + 1) * P],
                            O8[:, g, :, :],
                            start=(g == 0), stop=(g == 1),
                            perf_mode=DR,
                        )
                    tno = work.tile([P, 512], F32, tag="tno", bufs=3)
                    nc.vector.tensor_tensor(
                        out=tno, in0=pr, in1=zbcS, op=mybir.AluOpType.mult,
                    )
                    nc.vector.scalar_tensor_tensor(
                        out=ost_sb[:, g2, tt, isl_sl], in0=tno,
                        scalar=bpe[:, g2, tt : tt + 1],
                        in1=xq_sb[:, g2, tt, isl_sl],
                        op0=mybir.AluOpType.add, op1=mybir.AluOpType.add,
                    )
                    nc.gpsimd.dma_start(
                        out=OUT_d[:, g2, tt, isl_sl],
                        in_=ost_sb[:, g2, tt, isl_sl],
                    )

                for isl in range(2):
                    isl_sl = slice(isl * 512, (isl + 1) * 512)
                    o_ps = [
                        p3.tile([P, 512], F32, tag=f"o{ct}", bufs=1,
                                name=f"o{ct}_{isl}")
                        for ct in range(4)
                    ]
                    z_ps = p3.tile([P, 512], F32, tag="z", bufs=1)
                    # one-deep software pipeline: emit S/exp of jp+1 before
                    # the PV group of jp so the in-order PE stream never
                    # waits on the second exp of the current jp.
                    def s_group(jp, isl_sl=isl_sl):
                        ptp = work.tile([P, 2, 512], F8, tag="pt", bufs=3)
                        for t2 in range(2):
                            jt = 2 * jp + t2
                            sp = p3.tile([P, 512], F32, tag="s", bufs=2)
                            for g in range(2):
                                nc.tensor.matmul(
                                    sp,
                                    K8[:, g, :, jt * P : (jt + 1) * P],
                                    QO8[:, g, :, isl_sl],
                                    start=(g == 0), stop=(g == 1),
                                    perf_mode=DR,
                                )
                            nc.scalar.activation(
                                out=ptp[:, t2, :], in_=sp,
                                func=AF.Exp, scale=SCL, bias=neg2,
                            )
                        return ptp

                    cur_ptp = s_group(0)
                    for jp in range(NJP):
                        if jp + 1 < NJP:
                            nxt_ptp = s_group(jp + 1)
                        nc.tensor.matmul(
                            z_ps, on8, cur_ptp,
                            start=(jp == 0), stop=(jp == NJP - 1),
                            perf_mode=DR,
                        )
                        for ct in range(4):
                            nc.tensor.matmul(
                                o_ps[ct],
                                VT8[:, jp, :, ct * P : (ct + 1) * P],
                                cur_ptp,
                                start=(jp == 0), stop=(jp == NJP - 1),
                                perf_mode=DR,
                            )
                        if jp >= 1:
                            pop_deferred()
                        if jp + 1 < NJP:
                            cur_ptp = nxt_ptp
                    # x0.25 range guard on O/Z; 4/Z folded into zbcS.
                    # isl0: evac on DVE, zbc/recip/prs deferred into isl1's
                    # jp loop so the in-order PE stream never stalls on them.
                    zrow = work.tile([1, 512], F32R, tag="zrow", bufs=2)
                    nc.vector.tensor_scalar(
                        out=zrow, in0=z_ps[0:1, :], scalar1=0.25, scalar2=None,
                        op0=mybir.AluOpType.mult,
                    )
                    O8 = work.tile([P, 2, 2, 512], F8, tag="o8", bufs=2)
                    zbcS = work.tile([P, 512], F32, tag="zbs", bufs=2,
                                     name=f"zbcS{isl}")

                    def emit_zbc(isl=isl, zrow=zrow, zbcS=zbcS):
                        zbc_ps = p3.tile([P, 512], F32, tag="zb", bufs=1,
                                         name=f"zbc{isl}")
                        nc.tensor.matmul(zbc_ps, onq, zrow, start=True, stop=True)
                        nc.vector.reciprocal(out=zbcS, in_=zbc_ps)

                    if isl == 0:
                        for ct in range(4):
                            nc.vector.tensor_scalar(
                                out=O8[:, ct // 2, ct % 2, :], in0=o_ps[ct],
                                scalar1=0.25, scalar2=None,
                                op0=mybir.AluOpType.mult,
                            )
                        deferred.append(emit_zbc)
                        for ct in range(4):
                            deferred.append(
                                lambda ct=ct, O8=O8, zbcS=zbcS:
                                proj_epilogue(0, ct, O8, zbcS)
                            )
                    else:
                        for ct in range(4):
                            nc.scalar.activation(
                                out=O8[:, ct // 2, ct % 2, :], in_=o_ps[ct],
                                func=AF.Identity, scale=0.25,
                            )
                        emit_zbc()
                        for ct in range(4):
                            proj_epilogue(1, ct, O8, zbcS)
                while deferred:
                    pop_deferred()


    nc.compile()
    return nc


def _get_nc():
    if "nc" not in _cached:
        _cached["nc"] = _build_program()
    return _cached["nc"]


def _f8(a):
    return np.clip(np.ascontiguousarray(a, dtype=np.float32), -240, 240).astype(E4)


def _gt(v):
    """[C] -> [P, 2, 2] with channel c = p + 128*t + 256*g at [p, g, t]."""
    return np.ascontiguousarray(
        np.asarray(v, np.float32).reshape(2, 2, P).transpose(2, 0, 1)
    )


def _xprep(a2d, ncols):
    """[C, ncols] -> [P, 2, 2, ncols]."""
    return np.ascontiguousarray(
        a2d.reshape(2, 2, P, ncols).transpose(2, 0, 1, 3)
    )


def _wprep(w):
    """[Cout, Cin] -> lhsT layout [P, 2, 2, Cout] fp8 (ci = p+128t+256g)."""
    return _f8(np.asarray(w, np.float32).T.reshape(2, 2, P, C).transpose(2, 0, 1, 3))


def _make_in_maps(x, norm_gamma, norm_beta, wq, bq, wk, bk, wv, bv, wp, bp):
    gm = np.zeros((P, 2, 2, NGROUPS), np.float32)
    em = np.zeros((NGROUPS, 2, 2, P), np.float32)
    for g in range(2):
        for t2 in range(2):
            for p in range(P):
                grp = p // GSIZE + 8 * t2 + 16 * g
                gm[p, g, t2, grp] = 1.0
                em[grp, g, t2, p] = 1.0

    cpack = np.stack(
        [_gt(bq), _gt(bk), _gt(bv), _gt(bp), _gt(norm_gamma), _gt(norm_beta)],
        axis=-1,
    )

    common = {
        "w38": np.ascontiguousarray(
            np.stack([_wprep(wq), _wprep(wk), _wprep(wv)], axis=3)
        ),
        "wp8": _wprep(wp),
        "cpack": np.ascontiguousarray(cpack),
        "gmat": gm,
        "emat": em,
        "ones8": np.ones((P, 2, P), np.float32).astype(E4),
        "onesq": np.ones((1, P), np.float32),
    }

    in_maps = []
    for c in range(NCORES):
        b, qi = c // 4, c % 4
        xb = np.ascontiguousarray(np.asarray(x[b], dtype=np.float32).reshape(C, N))
        xp = np.concatenate([xb[:, qi * NQ :], xb[:, : qi * NQ]], axis=1)
        m = dict(common)
        m["xin8"] = _f8(_xprep(xp, N))
        m["xq"] = _xprep(xb[:, qi * NQ : (qi + 1) * NQ], NQ)
        in_maps.append(m)
    return in_maps


def _assemble(results):
    out = np.empty((B, C, N), np.float32)
    for c in range(NCORES):
        b, qi = c // 4, c % 4
        r = results[c]["out"]  # [P, 2, 2, NQ]
        out[b, :, qi * NQ : (qi + 1) * NQ] = (
            r.transpose(1, 2, 0, 3).reshape(C, NQ)
        )
    return out.reshape(B, C, HW, HW)


def _run(inputs, trace=False, trace_kwargs=None):
    nc = _get_nc()
    in_maps = _make_in_maps(**inputs)
    res = run_bass_kernel_spmd(
        nc, in_maps, list(range(NCORES)), trace=trace,
        **(trace_kwargs or {}),
    )
    return res


def kernel(**inputs):
    res = _run(inputs)
    return _assemble(res.results)


# revision 5
# speedup vs baseline: 1.0227x; 1.0227x over previous
"""AttnBlock (B=2, C=512, H=W=64) on 8 TRN2 NeuronCores — fp8 DoubleRow.

Sharding: core c handles batch b=c//4 and query-quarter q=c%4 (1024 of 4096
query positions). Keys/values are computed redundantly per core from the
full batch image; the key axis is host-permuted per core so the core's
query quarter occupies columns 0:1024 (softmax is permutation-invariant
over keys).

All matmuls run in fp8e4 (TRN FP8_EXP4, max 240) with DoubleRow perf mode:
each instruction contracts K=256 (two 128-row subtiles interleaved in the
free dim) in the same 512 cycles a bf16/f32r K=128 matmul takes — 2x.
Channel dim C=512 is stored pair-split as [128 p, 2 g, 2 t] with
c = p + 128*t + 256*g; a DR matmul contracts (p, t) for fixed g, and the
two g-groups accumulate in PSUM.

GroupNorm: bn_stats on the fp8 x (24 slices on DVE, 8 slices via ACT
Identity/Square accum_out), group reduce via one-hot matmuls; scale/shift
fold into the q/k/v weights on device (wq' = wq*diag(s), bias' = wq@t + b
via tiny DR matvecs), so no separate normalize pass exists. Softmax runs
unshifted with exp(s*C^-.5 - 2) to keep fp8 in range; Z comes from a
ones-stationary DR matmul accumulated alongside PV; O is normalized by
1/Z (broadcast via a K=1 outer-product matmul + full-width reciprocal)
during PSUM evacuation. The residual path stays exact fp32.
"""

import numpy as np
import ml_dtypes

import concourse.bass as bass
import concourse.tile as tile
from concourse import bacc, mybir
from concourse.bass_utils import run_bass_kernel_spmd

F32 = mybir.dt.float32
F32R = mybir.dt.float32r
F8 = mybir.dt.float8e4
E4 = ml_dtypes.float8_e4m3
DR = mybir.MatmulPerfMode.DoubleRow
AF = mybir.ActivationFunctionType

P = 128
C = 512
N = 4096          # H*W keys
NQ = 1024         # query columns per core
NS = 8            # 512-wide column slices of N
SPL = 6           # slices per (g,t) whose stats run on DVE (rest on ACT)
NJP = 16          # 256-wide key pair-tiles
B = 2
HW = 64
NGROUPS = 32
GSIZE = C // NGROUPS
EPS = 1e-5
SCL = float(C) ** -0.5
EBIAS = -2.0      # exp(s*SCL - 2): max logit ~5.5 -> exp(3.5)=33 << 240
TS = 64.0         # shift vector pre-scale for fp8 matvec
NCORES = 8

_cached = {}


def _build_program():
    nc = bacc.Bacc("TRN2", target_bir_lowering=False, debug=False)

    X8_d = nc.declare_dram_parameter("xin8", [P, 2, 2, N], F8, isOutput=False)
    W3_d = nc.declare_dram_parameter("w38", [P, 2, 2, 3, C], F8, isOutput=False)
    WP_d = nc.declare_dram_parameter("wp8", [P, 2, 2, C], F8, isOutput=False)
    # packed per-channel f32 consts: bq, bk, bv, bp, gamma, beta
    CP_d = nc.declare_dram_parameter("cpack", [P, 2, 2, 6], F32, isOutput=False)
    G_d = nc.declare_dram_parameter("gmat", [P, 2, 2, NGROUPS], F32, isOutput=False)
    E_d = nc.declare_dram_parameter("emat", [NGROUPS, 2, 2, P], F32, isOutput=False)
    ON8_d = nc.declare_dram_parameter("ones8", [P, 2, P], F8, isOutput=False)
    ONQ_d = nc.declare_dram_parameter("onesq", [1, P], F32R, isOutput=False)
    XQ_d = nc.declare_dram_parameter("xq", [P, 2, 2, NQ], F32, isOutput=False)
    OUT_d = nc.declare_dram_parameter("out", [P, 2, 2, NQ], F32, isOutput=True)

    with tile.TileContext(nc) as tc:
        with (
            tc.tile_pool(name="big", bufs=1) as big,
            tc.tile_pool(name="consts", bufs=1) as consts,
            tc.tile_pool(name="stat", bufs=1) as stat,
            tc.tile_pool(name="work", bufs=1) as work,
        ):
            X8 = big.tile([P, 2, 2, N], F8)
            K8 = big.tile([P, 2, 2, N], F8)
            VT8 = big.tile([P, NJP, 2, C], F8)
            QO8 = big.tile([P, 2, 2, NQ], F8)
            xq_sb = big.tile([P, 2, 2, NQ], F32)
            ost_sb = big.tile([P, 2, 2, NQ], F32)

            w38 = consts.tile([P, 2, 2, 3, C], F8)
            w3f = consts.tile([P, 2, 2, 3, C], F8)
            wp8 = consts.tile([P, 2, 2, C], F8)
            cpk = consts.tile([P, 2, 2, 6], F32)
            gmat = consts.tile([P, 2, 2, NGROUPS], F32)
            emat = consts.tile([NGROUPS, 2, 2, P], F32)
            on8 = consts.tile([P, 2, P], F8)
            onq = consts.tile([1, P], F32R)

            # preload ACT tables (Identity/Square/Sqrt/Exp) while DMA runs
            dummy = stat.tile([1, 2], F32)
            nc.vector.memset(dummy, 1.0)
            dscr = stat.tile([1, 2], F32)
            for fn in (AF.Identity, AF.Square):
                nc.scalar.activation(out=dscr, in_=dummy, func=fn)

            # x first: groupnorm stats are the serial head of the kernel
            for g in range(2):
                for t2 in range(2):
                    nc.sync.dma_start(out=X8[:, g, t2, :], in_=X8_d[:, g, t2, :])
            for t_ in (
                (w38, W3_d), (cpk, CP_d),
                (gmat, G_d), (emat, E_d), (on8, ON8_d), (onq, ONQ_d),
                (wp8, WP_d),
            ):
                nc.gpsimd.dma_start(out=t_[0], in_=t_[1][:])
            nc.gpsimd.dma_start(out=xq_sb, in_=XQ_d[:])

            bqh = cpk[:, :, :, 0]
            bkh = cpk[:, :, :, 1]
            bvh = cpk[:, :, :, 2]
            bph = cpk[:, :, :, 3]
            gam_sb = cpk[:, :, :, 4]
            bet_sb = cpk[:, :, :, 5]

            # ---------------- Phase 1: group-norm statistics ----------------
            # 24 slices via DVE bn_stats, 8 slices via ACT accum (sum, sumsq)
            bnst = stat.tile([P, 2, 2, SPL, 6], F32)
            asum = stat.tile([P, 2, 2, 2, 2], F32)
            ascr = stat.tile([P, 2, 512], F8)
            mex = stat.tile([P, 2, 2, 2], F32)
            for g in range(2):
                for t2 in range(2):
                    for s in range(SPL):
                        nc.vector.bn_stats(
                            out=bnst[:, g, t2, s, :],
                            in_=X8[:, g, t2, s * 512 : (s + 1) * 512],
                        )
                    nc.vector.bn_aggr(
                        out=mex[:, g, t2, :], in_=bnst[:, g, t2, :, :]
                    )
                    for si in range(2):
                        sl = slice((SPL + si) * 512, (SPL + si + 1) * 512)
                        nc.scalar.activation(
                            out=ascr[:, 0, :], in_=X8[:, g, t2, sl],
                            func=AF.Identity,
                            accum_out=asum[:, g, t2, si, 0:1],
                        )
                        nc.scalar.activation(
                            out=ascr[:, 1, :], in_=X8[:, g, t2, sl],
                            func=AF.Square,
                            accum_out=asum[:, g, t2, si, 1:2],
                        )
            # preload the Sqrt table now: the load overlaps the aggr/mexp
            # DVE work instead of sitting on the group-reduce critical path
            nc.scalar.activation(out=dscr, in_=dummy, func=AF.Sqrt)

            # mexp[...,0] = mean over 4096, mexp[...,1] = E[x^2] over 4096
            W_DVE = SPL / float(NS)
            astot = stat.tile([P, 2, 2, 2], F32)
            nc.vector.tensor_add(
                out=astot, in0=asum[:, :, :, 0, :], in1=asum[:, :, :, 1, :]
            )
            mexp = stat.tile([P, 2, 2, 2], F32)
            t1s = stat.tile([P, 2, 2], F32)
            nc.vector.tensor_scalar(
                out=t1s, in0=mex[:, :, :, 0], scalar1=W_DVE, scalar2=None,
                op0=mybir.AluOpType.mult,
            )
            nc.vector.scalar_tensor_tensor(
                out=mexp[:, :, :, 0], in0=astot[:, :, :, 0],
                scalar=1.0 / float(N), in1=t1s,
                op0=mybir.AluOpType.mult, op1=mybir.AluOpType.add,
            )
            nc.vector.tensor_tensor(
                out=t1s, in0=mex[:, :, :, 0], in1=mex[:, :, :, 0],
                op=mybir.AluOpType.mult,
            )
            nc.vector.tensor_add(out=t1s, in0=t1s, in1=mex[:, :, :, 1])
            nc.vector.tensor_scalar(
                out=t1s, in0=t1s, scalar1=W_DVE, scalar2=None,
                op0=mybir.AluOpType.mult,
            )
            nc.vector.scalar_tensor_tensor(
                out=mexp[:, :, :, 1], in0=astot[:, :, :, 1],
                scalar=1.0 / float(N), in1=t1s,
                op0=mybir.AluOpType.mult, op1=mybir.AluOpType.add,
            )

            scale_c = stat.tile([P, 2, 2], F32)
            shift_c = stat.tile([P, 2, 2], F32)
            tv8 = stat.tile([P, 2, 2, 16], F8)
            bv8 = stat.tile([P, 2, 2, 16], F8)
            bqe = stat.tile([P, 2, 2], F32)
            bke = stat.tile([P, 2, 2], F32)
            bve = stat.tile([P, 2, 2], F32)
            bpe = stat.tile([P, 2, 2], F32)
            neg2 = stat.tile([P, 1], F32)
            nc.vector.memset(neg2, EBIAS)

            with tc.tile_pool(name="psum_p1", bufs=1, space="PSUM") as p1:
                gs_ps = p1.tile([NGROUPS, 2], F32, tag="gs")
                kk = 0
                for g in range(2):
                    for t2 in range(2):
                        nc.tensor.matmul(
                            gs_ps, gmat[:, g, t2, :], mexp[:, g, t2, :],
                            start=(kk == 0), stop=(kk == 3),
                        )
                        kk += 1
                gsb = stat.tile([NGROUPS, 2], F32)
                nc.vector.tensor_copy(out=gsb, in_=gs_ps)
                gmr = stat.tile([NGROUPS, 2], F32)
                gtmp = stat.tile([NGROUPS, 2], F32)
                nc.vector.tensor_scalar(
                    out=gmr[:, 0:1], in0=gsb[:, 0:1], scalar1=1.0 / GSIZE,
                    scalar2=None, op0=mybir.AluOpType.mult,
                )
                nc.vector.tensor_scalar(
                    out=gtmp[:, 0:1], in0=gsb[:, 1:2], scalar1=1.0 / GSIZE,
                    scalar2=None, op0=mybir.AluOpType.mult,
                )
                nc.vector.tensor_tensor(
                    out=gtmp[:, 1:2], in0=gmr[:, 0:1], in1=gmr[:, 0:1],
                    op=mybir.AluOpType.mult,
                )
                nc.vector.tensor_sub(
                    out=gtmp[:, 0:1], in0=gtmp[:, 0:1], in1=gtmp[:, 1:2]
                )
                eps_sb = stat.tile([NGROUPS, 1], F32)
                nc.vector.memset(eps_sb, EPS)
                nc.scalar.activation(
                    out=gtmp[:, 0:1], in_=gtmp[:, 0:1],
                    func=AF.Sqrt, bias=eps_sb,
                )
                nc.vector.reciprocal(out=gmr[:, 1:2], in_=gtmp[:, 0:1])
                mc = stat.tile([P, 2, 2, 2], F32)
                ms_list = []
                for g in range(2):
                    for t2 in range(2):
                        ms_ps = p1.tile(
                            [P, 2], F32, tag="ms", bufs=4, name=f"ms{g}{t2}"
                        )
                        nc.tensor.matmul(
                            ms_ps, emat[:, g, t2, :], gmr, start=True, stop=True
                        )
                        ms_list.append((g, t2, ms_ps))
                for g, t2, ms_ps in ms_list:
                    nc.vector.tensor_copy(out=mc[:, g, t2, :], in_=ms_ps)
                nc.vector.tensor_tensor(
                    out=scale_c, in0=mc[:, :, :, 1], in1=gam_sb,
                    op=mybir.AluOpType.mult,
                )
                nc.vector.tensor_tensor(
                    out=shift_c, in0=mc[:, :, :, 0], in1=scale_c,
                    op=mybir.AluOpType.mult,
                )
                nc.vector.tensor_sub(out=shift_c, in0=bet_sb, in1=shift_c)

                # fold norm scale into q/k/v weights (wq on DVE first so the
                # q-projection can start; wk/wv on ACT in parallel)
                nc.vector.tensor_scalar(
                    out=tv8[:, :, :, 0], in0=shift_c, scalar1=TS, scalar2=None,
                    op0=mybir.AluOpType.mult,
                )
                for g in range(2):
                    for t2 in range(2):
                        sc1 = scale_c[:, g, t2 : t2 + 1]
                        if t2 == 0:
                            nc.vector.tensor_scalar(
                                out=w3f[:, g, t2, :, :], in0=w38[:, g, t2, :, :],
                                scalar1=sc1, scalar2=None,
                                op0=mybir.AluOpType.mult,
                            )
                        else:
                            nc.scalar.activation(
                                out=w3f[:, g, t2, :, :], in_=w38[:, g, t2, :, :],
                                func=AF.Identity, scale=sc1,
                            )

                # effective biases: bX_eff = wX @ shift + bX (tiny DR matvecs)
                def matvec(wi, rhs8, bh, bdst):
                    for ct in range(4):
                        g2, tt = ct // 2, ct % 2
                        be_ps = p1.tile([P, 1], F32, tag="bias", bufs=3)
                        for g in range(2):
                            lhs = (
                                wp8[:, g, :, ct * P : (ct + 1) * P]
                                if wi == 3
                                else w38[:, g, :, wi, ct * P : (ct + 1) * P]
                            )
                            nc.tensor.matmul(
                                be_ps, lhs, rhs8[:, g, :, 0:1],
                                start=(g == 0), stop=(g == 1),
                                perf_mode=DR,
                            )
                        nc.vector.tensor_scalar(
                            out=bdst[:, g2, tt : tt + 1], in0=be_ps,
                            scalar1=1.0 / TS, scalar2=bh[:, g2, tt : tt + 1],
                            op0=mybir.AluOpType.mult, op1=mybir.AluOpType.add,
                        )

                matvec(0, tv8, bqh, bqe)
                matvec(1, tv8, bkh, bke)


            # ---------------- Phase 2: q/k/v projections --------------------
            ev = {"n": 0}

            def evac(dst, src_ps, bias_ap=None, force_dve=False):
                use_act = (ev["n"] % 2 == 0) and not force_dve
                ev["n"] += 1
                if use_act:
                    nc.scalar.activation(
                        out=dst, in_=src_ps, func=AF.Identity,
                        bias=bias_ap if bias_ap is not None else 0.0,
                    )
                elif bias_ap is None:
                    nc.vector.tensor_copy(out=dst, in_=src_ps)
                else:
                    nc.vector.tensor_scalar(
                        out=dst, in0=src_ps, scalar1=bias_ap, scalar2=None,
                        op0=mybir.AluOpType.add,
                    )

            with tc.tile_pool(name="psum2", bufs=1, space="PSUM") as p2:

                def late_biases():
                    matvec2(2, tv8, bvh, bve)
                    nc.vector.tensor_scalar(
                        out=bv8[:, :, :, 0], in0=bve, scalar1=TS, scalar2=None,
                        op0=mybir.AluOpType.mult,
                    )
                    matvec2(3, bv8, bph, bpe)

                def matvec2(wi, rhs8, bh, bdst):
                    for ct in range(4):
                        g2, tt = ct // 2, ct % 2
                        be_ps = p2.tile([P, 1], F32, tag="bias", bufs=2)
                        for g in range(2):
                            lhs = (
                                wp8[:, g, :, ct * P : (ct + 1) * P]
                                if wi == 3
                                else w38[:, g, :, wi, ct * P : (ct + 1) * P]
                            )
                            nc.tensor.matmul(
                                be_ps, lhs, rhs8[:, g, :, 0:1],
                                start=(g == 0), stop=(g == 1),
                                perf_mode=DR,
                            )
                        nc.vector.tensor_scalar(
                            out=bdst[:, g2, tt : tt + 1], in0=be_ps,
                            scalar1=1.0 / TS, scalar2=bh[:, g2, tt : tt + 1],
                            op0=mybir.AluOpType.mult, op1=mybir.AluOpType.add,
                        )

                for s in range(NS):
                    sl = slice(s * 512, (s + 1) * 512)
                    if s < 2:
                        for ct in range(4):
                            g2, tt = ct // 2, ct % 2
                            qp = p2.tile([P, 512], F32, tag="acc", bufs=3)
                            for g in range(2):
                                nc.tensor.matmul(
                                    qp,
                                    w3f[:, g, :, 0, ct * P : (ct + 1) * P],
                                    X8[:, g, :, sl],
                                    start=(g == 0), stop=(g == 1),
                                    perf_mode=DR,
                                )
                            evac(QO8[:, g2, tt, sl], qp, bqe[:, g2, tt : tt + 1])
                    for jt4 in range(4):
                        jt = s * 4 + jt4
                        jb = slice(jt * P, (jt + 1) * P)
                        vp = p2.tile([P, 512], F32, tag="acc", bufs=3)
                        for g in range(2):
                            nc.tensor.matmul(
                                vp, X8[:, g, :, jb], w3f[:, g, :, 2, :],
                                start=(g == 0), stop=(g == 1),
                                perf_mode=DR,
                            )
                        evac(VT8[:, jt // 2, jt % 2, :], vp,
                             force_dve=(s == NS - 1))
                    for ct in range(4):
                        g2, tt = ct // 2, ct % 2
                        kp = p2.tile([P, 512], F32, tag="acc", bufs=3)
                        for g in range(2):
                            nc.tensor.matmul(
                                kp,
                                w3f[:, g, :, 1, ct * P : (ct + 1) * P],
                                X8[:, g, :, sl],
                                start=(g == 0), stop=(g == 1),
                                perf_mode=DR,
                            )
                        evac(K8[:, g2, tt, sl], kp, bke[:, g2, tt : tt + 1],
                             force_dve=(s == NS - 1))
                    if s == 0:
                        late_biases()
                        nc.scalar.activation(out=dscr, in_=dummy, func=AF.Exp)

            # ---------------- Phase 3: attention -----------------------------
            # isl 0's output projection + epilogue are interleaved into
            # isl 1's jp loop (PSUM tag "zb" hosts zbc then the pr tiles).
            deferred = []

            def pop_deferred():
                if deferred:
                    deferred.pop(0)()

            with tc.tile_pool(name="psum3", bufs=1, space="PSUM") as p3:

                def proj_epilogue(isl, ct, O8, zbcS, p3=p3):
                    g2, tt = ct // 2, ct % 2
                    isl_sl = slice(isl * 512, (isl + 1) * 512)
                    tag = "zb" if isl == 0 else f"o{ct}"
                    pr = p3.tile([P, 512], F32, tag=tag, bufs=1, name=f"pr{isl}{ct}")
                    for g in range(2):
                        nc.tensor.matmul(
                            pr,
                            wp8[:, g, :, ct * P : (ct + 1) * P],
                            O8[:, g, :, :],
                            start=(g == 0), stop=(g == 1),
                            perf_mode=DR,
                        )
                    tno = work.tile([P, 512], F32, tag="tno", bufs=3)
                    nc.vector.tensor_tensor(
                        out=tno, in0=pr, in1=zbcS, op=mybir.AluOpType.mult,
                    )
                    nc.vector.scalar_tensor_tensor(
                        out=ost_sb[:, g2, tt, isl_sl], in0=tno,
                        scalar=bpe[:, g2, tt : tt + 1],
                        in1=xq_sb[:, g2, tt, isl_sl],
                        op0=mybir.AluOpType.add, op1=mybir.AluOpType.add,
                    )
                    nc.gpsimd.dma_start(
                        out=OUT_d[:, g2, tt, isl_sl],
                        in_=ost_sb[:, g2, tt, isl_sl],
                    )

                for isl in range(2):
                    isl_sl = slice(isl * 512, (isl + 1) * 512)
                    o_ps = [
                        p3.tile([P, 512], F32, tag=f"o{ct}", bufs=1,
                                name=f"o{ct}_{isl}")
                        for ct in range(4)
                    ]
                    z_ps = p3.tile([P, 512], F32, tag="z", bufs=1)
                    # one-deep software pipeline: emit S/exp of jp+1 before
                    # the PV group of jp so the in-order PE stream never
                    # waits on the second exp of the current jp.
                    def s_group(jp, isl_sl=isl_sl):
                        ptp = work.tile([P, 2, 512], F8, tag="pt", bufs=3)
                        for t2 in range(2):
                            jt = 2 * jp + t2
                            sp = p3.tile([P, 512], F32, tag="s", bufs=2)
                            for g in range(2):
                                nc.tensor.matmul(
                                    sp,
                                    K8[:, g, :, jt * P : (jt + 1) * P],
                                    QO8[:, g, :, isl_sl],
                                    start=(g == 0), stop=(g == 1),
                                    perf_mode=DR,
                                )
                            nc.scalar.activation(
                                out=ptp[:, t2, :], in_=sp,
                                func=AF.Exp, scale=SCL, bias=neg2,
                            )
                        return ptp

                    cur_ptp = s_group(0)
                    for jp in range(NJP):
                        if jp + 1 < NJP:
                            nxt_ptp = s_group(jp + 1)
                        nc.tensor.matmul(
                            z_ps, on8, cur_ptp,
                            start=(jp == 0), stop=(jp == NJP - 1),
                            perf_mode=DR,
                        )
                        for ct in range(4):
                            nc.tensor.matmul(
                                o_ps[ct],
                                VT8[:, jp, :, ct * P : (ct + 1) * P],
                                cur_ptp,
                                start=(jp == 0), stop=(jp == NJP - 1),
                                perf_mode=DR,
                            )
                        if jp >= 1:
                            pop_deferred()
                        if jp + 1 < NJP:
                            cur_ptp = nxt_ptp
                    # x0.25 range guard on O/Z; 4/Z folded into zbcS.
                    # isl0: evac on DVE, zbc/recip/prs deferred into isl1's
                    # jp loop so the in-order PE stream never stalls on them.
                    zrow = work.tile([1, 512], F32R, tag="zrow", bufs=2)
                    nc.vector.tensor_scalar(
                        out=zrow, in0=z_ps[0:1, :], scalar1=0.25, scalar2=None,
                        op0=mybir.AluOpType.mult,
                    )
                    O8 = work.tile([P, 2, 2, 512], F8, tag="o8", bufs=2)
                    zbcS = work.tile([P, 512], F32, tag="zbs", bufs=2,
                                     name=f"zbcS{isl}")

                    def emit_zbc(isl=isl, zrow=zrow, zbcS=zbcS):
                        zbc_ps = p3.tile([P, 512], F32, tag="zb", bufs=1,
                                         name=f"zbc{isl}")
                        nc.tensor.matmul(zbc_ps, onq, zrow, start=True, stop=True)
                        nc.vector.reciprocal(out=zbcS, in_=zbc_ps)

                    if isl == 0:
                        for ct in range(4):
                            nc.vector.tensor_scalar(
                                out=O8[:, ct // 2, ct % 2, :], in0=o_ps[ct],
                                scalar1=0.25, scalar2=None,
                                op0=mybir.AluOpType.mult,
                            )
                        deferred.append(emit_zbc)
                        for ct in range(4):
                            deferred.append(
                                lambda ct=ct, O8=O8, zbcS=zbcS:
                                proj_epilogue(0, ct, O8, zbcS)
                            )
                    else:
                        for ct in range(4):
                            nc.scalar.activation(
                                out=O8[:, ct // 2, ct % 2, :], in_=o_ps[ct],
                                func=AF.Identity, scale=0.25,
                            )
                        emit_zbc()
                        for ct in range(4):
                            proj_epilogue(1, ct, O8, zbcS)
                while deferred:
                    pop_deferred()


    nc.compile()
    return nc


def _get_nc():
    if "nc" not in _cached:
        _cached["nc"] = _build_program()
    return _cached["nc"]


def _f8(a):
    return np.clip(np.ascontiguousarray(a, dtype=np.float32), -240, 240).astype(E4)


def _gt(v):
    """[C] -> [P, 2, 2] with channel c = p + 128*t + 256*g at [p, g, t]."""
    return np.ascontiguousarray(
        np.asarray(v, np.float32).reshape(2, 2, P).transpose(2, 0, 1)
    )


def _xprep(a2d, ncols):
    """[C, ncols] -> [P, 2, 2, ncols]."""
    return np.ascontiguousarray(
        a2d.reshape(2, 2, P, ncols).transpose(2, 0, 1, 3)
    )


def _wprep(w):
    """[Cout, Cin] -> lhsT layout [P, 2, 2, Cout] fp8 (ci = p+128t+256g)."""
    return _f8(np.asarray(w, np.float32).T.reshape(2, 2, P, C).transpose(2, 0, 1, 3))


def _make_in_maps(x, norm_gamma, norm_beta, wq, bq, wk, bk, wv, bv, wp, bp):
    gm = np.zeros((P, 2, 2, NGROUPS), np.float32)
    em = np.zeros((NGROUPS, 2, 2, P), np.float32)
    for g in range(2):
        for t2 in range(2):
            for p in range(P):
                grp = p // GSIZE + 8 * t2 + 16 * g
                gm[p, g, t2, grp] = 1.0
                em[grp, g, t2, p] = 1.0

    cpack = np.stack(
        [_gt(bq), _gt(bk), _gt(bv), _gt(bp), _gt(norm_gamma), _gt(norm_beta)],
        axis=-1,
    )

    common = {
        "w38": np.ascontiguousarray(
            np.stack([_wprep(wq), _wprep(wk), _wprep(wv)], axis=3)
        ),
        "wp8": _wprep(wp),
        "cpack": np.ascontiguousarray(cpack),
        "gmat": gm,
        "emat": em,
        "ones8": np.ones((P, 2, P), np.float32).astype(E4),
        "onesq": np.ones((1, P), np.float32),
    }

    in_maps = []
    for c in range(NCORES):
        b, qi = c // 4, c % 4
        xb = np.ascontiguousarray(np.asarray(x[b], dtype=np.float32).reshape(C, N))
        xp = np.concatenate([xb[:, qi * NQ :], xb[:, : qi * NQ]], axis=1)
        m = dict(common)
        m["xin8"] = _f8(_xprep(xp, N))
        m["xq"] = _xprep(xb[:, qi * NQ : (qi + 1) * NQ], NQ)
        in_maps.append(m)
    return in_maps


def _assemble(results):
    out = np.empty((B, C, N), np.float32)
    for c in range(NCORES):
        b, qi = c // 4, c % 4
        r = results[c]["out"]  # [P, 2, 2, NQ]
        out[b, :, qi * NQ : (qi + 1) * NQ] = (
            r.transpose(1, 2, 0, 3).reshape(C, NQ)
        )
    return out.reshape(B, C, HW, HW)


def _run(inputs, trace=False, trace_kwargs=None):
    nc = _get_nc()
    in_maps = _make_in_maps(**inputs)
    res = run_bass_kernel_spmd(
        nc, in_maps, list(range(NCORES)), trace=trace,
        **(trace_kwargs or {}),
    )
    return res


def kernel(**inputs):
    res = _run(inputs)
    return _assemble(res.results)
